# revision 8
# baseline (speedup 1.0000x reference)
"""Trainium2 Bass kernel for nn_KG_EdgeAtt_new (sparse windowed attention).

Sharding: pure data-parallel over batch B=32 across 8 NeuronCores (4
conversations per core). Weights replicated. Host marshals inputs
(transposes / bf16 casts / window+length masks); all FLOPs run on device.

Math (per batch b):
  semantic:   S = W_sem-transform of node_features; cos(nf_j, S_k);
              score = 1 - acos(clip(cos))/pi; windowed softmax -> alphas_sem
  contextual: A_n = K_n @ W_con (per knowledge slot n); cos(K_nj, A_nk)
              (the anew affinity scale is strictly positive so it cancels
              exactly in cosine similarity -> anew is mathematically dead);
              alphas_con = 10 * sum_n |cos| (windowed)
  out = 0.5*alphas_sem + 0.5*alphas_con, masked.
"""

import sys

sys.path.insert(0, "/opt/trn_rl_repo")

import math
from contextlib import ExitStack

import ml_dtypes
import numpy as np

import concourse.bass as bass
import concourse.bacc as bacc
import concourse.mybir as mybir
import concourse.tile as tile
from concourse.bass import ds, ts
from concourse.bass_utils import run_bass_kernel_spmd

BF = mybir.dt.bfloat16
F32 = mybir.dt.float32
AF = mybir.ActivationFunctionType
OP = mybir.AluOpType
AX = mybir.AxisListType

B, L, G, N, D = 32, 110, 512, 40, 300
NCORES = 8
BPC = B // NCORES  # 4
WP, WF = 10, 10
CLIP = 1.0 - 1e-6
NG = 4                      # knowledge slots per matmul group (free dim 440)
NGRP = N // NG              # 10
BL = BPC * L                # 440
DT = [128, 128, 44]         # 300 split into partition tiles
GT = [128, 128, 128, 128]   # 512 split
P = 128
NEG = 1.0e4                 # masked-logit offset (exp(-1e4) == 0 in f32)

# acos(x) ~= sqrt(1-x) * (a0 + a1 x + a2 x^2 + a3 x^3), x in [0,1]  (A&S 4.4.45)
A0, A1, A2, A3 = 1.5707288, -0.2121144, 0.0742610, -0.0187293


def _build_nc():
    nc = bacc.Bacc("TRN2", target_bir_lowering=False, debug=False, num_devices=NCORES)
    kT = nc.declare_dram_parameter("kT", [BPC, D, N, L], BF, isOutput=False)
    nfT = nc.declare_dram_parameter("nfT", [G, BPC, L], BF, isOutput=False)
    nf = nc.declare_dram_parameter("nf", [BPC, L, G], F32, isOutput=False)
    wsemT = nc.declare_dram_parameter("wsemT", [G, G], BF, isOutput=False)
    wcon = nc.declare_dram_parameter("wcon", [D, D], BF, isOutput=False)
    fmask = nc.declare_dram_parameter("fmask", [BPC, L, L], F32, isOutput=False)
    out = nc.declare_dram_parameter("out", [BPC, L, L], F32, isOutput=True)

    with tile.TileContext(nc) as tc, ExitStack() as ctx:
        _emit(ctx, tc, nc, kT, nfT, nf, wsemT, wcon, fmask, out)
    nc.compile()
    return nc


def _emit(ctx, tc, nc, kT, nfT, nf, wsemT, wcon, fmask, out):
    consts = ctx.enter_context(tc.tile_pool(name="consts", bufs=1))

    ones_bf = consts.tile([P, P], BF, tag="ones")
    nc.gpsimd.memset(ones_bf[:], 1.0)

    wsem_sb = []
    for i in range(4):
        t = consts.tile([P, G], BF, tag=f"wsem{i}")
        nc.sync.dma_start(out=t[:], in_=wsemT[ts(i, P), :])
        wsem_sb.append(t)
    wcon_sb = []
    for i, d_ in enumerate(DT):
        t = consts.tile([P, D], BF, tag=f"wcon{i}")
        nc.sync.dma_start(out=t[:d_], in_=wcon[ds(i * 128, d_), :])
        wcon_sb.append(t)
    nfT_sb = []
    for i in range(4):
        t = consts.tile([P, BL], BF, tag=f"nfT{i}")
        nc.sync.dma_start(out=t[:], in_=nfT[ts(i, P)].rearrange("g b l -> g (b l)"))
        nfT_sb.append(t)
    fm_sb, fneg_sb = [], []
    for b in range(BPC):
        t = consts.tile([L, L], F32, tag=f"fm{b}")
        nc.sync.dma_start(out=t[:], in_=fmask[b])
        fm_sb.append(t)
        u = consts.tile([L, L], F32, tag=f"fn{b}")
        nc.vector.tensor_scalar(out=u[:], in0=t[:], scalar1=NEG, scalar2=-NEG,
                                op0=OP.mult, op1=OP.add)
        fneg_sb.append(u)

    # ---------------- semantic head: S_T, norms, num, cos ----------------
    sem = ctx.enter_context(tc.tile_pool(name="sem", bufs=1))
    cos_sb = []
    with tc.tile_pool(name="psS", bufs=4, space="PSUM") as psS, \
         tc.tile_pool(name="psNs", bufs=1, space="PSUM") as psNs, \
         tc.tile_pool(name="psM", bufs=2, space="PSUM") as psM:
        s_ps = []
        for gt in range(4):
            pt = psS.tile([P, BL], F32, tag="sps")
            for tt_ in range(4):
                nc.tensor.matmul(pt[:], lhsT=wsem_sb[tt_][:, ts(gt, P)],
                                 rhs=nfT_sb[tt_][:], start=(tt_ == 0), stop=(tt_ == 3))
            s_ps.append(pt)
        scp, ssq = [], []
        for gt in range(4):
            c = consts.tile([P, BL], BF, tag=f"scp{gt}")
            if gt % 2 == 0:
                nc.scalar.copy(out=c[:], in_=s_ps[gt][:])
            else:
                nc.vector.tensor_copy(c[:], s_ps[gt][:])
            scp.append(c)
            q = sem.tile([P, BL], BF, tag=f"ssq{gt}")
            nc.vector.tensor_mul(q[:], c[:], c[:])
            ssq.append(q)
        pn = psNs.tile([P, BL], F32, tag="pns")
        for gt in range(4):
            nc.tensor.matmul(pn[:], lhsT=ones_bf[:], rhs=ssq[gt][:],
                             start=(gt == 0), stop=(gt == 3))
        rna_f = sem.tile([P, BL], F32, tag="rnaf")
        nc.vector.reciprocal(rna_f[:], pn[:])
        rna = consts.tile([P, BL], F32, tag="rna")
        nc.scalar.sqrt(rna[:], rna_f[:])

        # nf row norms (natural layout, ACT square+accum)
        nfb = sem.tile([L, BPC * G], F32, tag="nfb")
        nc.sync.dma_start(out=nfb[:].rearrange("l (b g) -> l b g", b=BPC),
                          in_=nf.rearrange("b l g -> l b g"))
        rnf_sb = []
        for b in range(BPC):
            sc = sem.tile([L, G], F32, tag=f"nfsq{b}")
            acc1 = sem.tile([L, 1], F32, tag=f"nfacc{b}")
            nc.scalar.activation(sc[:], nfb[:, ts(b, G)], AF.Square, accum_out=acc1[:])
            rn1 = sem.tile([L, 1], F32, tag=f"rn1{b}")
            nc.vector.reciprocal(rn1[:], acc1[:])
            rnf = consts.tile([L, 1], F32, tag=f"rnf{b}")
            nc.scalar.sqrt(rnf[:], rn1[:])
            rnf_sb.append(rnf)

        for b in range(BPC):
            pm = psM.tile([L, L], F32, tag="pm")
            for gt in range(4):
                nc.tensor.matmul(pm[:], lhsT=nfT_sb[gt][:, ts(b, L)],
                                 rhs=scp[gt][:, ts(b, L)], start=(gt == 0), stop=(gt == 3))
            c1 = sem.tile([L, L], F32, tag="cosr")
            nc.vector.tensor_scalar(out=c1[:], in0=pm[:], scalar1=rnf_sb[b][:],
                                    scalar2=None, op0=OP.mult)
            cz = consts.tile([L, L], F32, tag=f"cos{b}")
            nc.vector.tensor_mul(cz[:], c1[:], rna[:L, ts(b, L)])
            cos_sb.append(cz)

    # ---------------- contextual branch ----------------
    tc.strict_bb_all_engine_barrier()
    kp = ctx.enter_context(tc.tile_pool(name="kp", bufs=6))
    ap = ctx.enter_context(tc.tile_pool(name="ap", bufs=6))
    sq = ctx.enter_context(tc.tile_pool(name="sq", bufs=6))
    kh = ctx.enter_context(tc.tile_pool(name="kh", bufs=6))
    rp = ctx.enter_context(tc.tile_pool(name="rp", bufs=2))
    cp = ctx.enter_context(tc.tile_pool(name="cp", bufs=3))
    accp = ctx.enter_context(tc.tile_pool(name="accp", bufs=1))
    semp = ctx.enter_context(tc.tile_pool(name="semp", bufs=2))
    psA = ctx.enter_context(tc.tile_pool(name="psA", bufs=3, space="PSUM"))
    psN = ctx.enter_context(tc.tile_pool(name="psN", bufs=2, space="PSUM"))
    psC = ctx.enter_context(tc.tile_pool(name="psC", bufs=3, space="PSUM"))

    for b in range(BPC):
        acc = accp.tile([L, NG * L], F32, tag=f"acc{b}")
        nc.gpsimd.memset(acc[:], 0.0)
        for g in range(NGRP):
            n0 = g * NG
            kts = []
            for i, d_ in enumerate(DT):
                t = kp.tile([P, NG * L], BF, tag="kt")
                nc.sync.dma_start(
                    out=t[:d_],
                    in_=kT[b, ds(i * 128, d_), ds(n0, NG), :].rearrange("d n l -> d (n l)"))
                kts.append(t)
            aps = []
            for ti, mt in enumerate(DT):
                pa = psA.tile([P, NG * L], F32, tag="pa")
                for si, st in enumerate(DT):
                    nc.tensor.matmul(pa[:mt], lhsT=wcon_sb[si][:st, ds(ti * 128, mt)],
                                     rhs=kts[si][:st], start=(si == 0), stop=(si == 2))
                aps.append(pa)
            acps = []
            for ti, mt in enumerate(DT):
                c = ap.tile([P, NG * L], BF, tag="ac")
                if ti == 2:
                    nc.vector.tensor_copy(c[:mt], aps[ti][:mt])
                else:
                    nc.scalar.copy(out=c[:mt], in_=aps[ti][:mt])
                acps.append(c)
            ksqs, asqs = [], []
            for ti, d_ in enumerate(DT):
                q = sq.tile([P, NG * L], BF, tag="ksq")
                nc.vector.tensor_mul(q[:d_], kts[ti][:d_], kts[ti][:d_])
                ksqs.append(q)
                q2 = sq.tile([P, NG * L], BF, tag="asq")
                nc.vector.tensor_mul(q2[:d_], acps[ti][:d_], acps[ti][:d_])
                asqs.append(q2)
            pk = psN.tile([P, NG * L], F32, tag="pn")
            for si, st in enumerate(DT):
                nc.tensor.matmul(pk[:], lhsT=ones_bf[:st, :], rhs=ksqs[si][:st],
                                 start=(si == 0), stop=(si == 2))
            pan = psN.tile([P, NG * L], F32, tag="pn")
            for si, st in enumerate(DT):
                nc.tensor.matmul(pan[:], lhsT=ones_bf[:st, :], rhs=asqs[si][:st],
                                 start=(si == 0), stop=(si == 2))
            rkf = rp.tile([P, NG * L], F32, tag="rkf")
            nc.vector.reciprocal(rkf[:], pk[:])
            rk = rp.tile([P, NG * L], BF, tag="rk")
            nc.scalar.sqrt(rk[:], rkf[:])
            raf = rp.tile([P, NG * L], F32, tag="raf")
            nc.vector.reciprocal(raf[:], pan[:])
            ra = rp.tile([P, NG * L], F32, tag="ra")
            nc.scalar.sqrt(ra[:], raf[:])
            khs = []
            for ti, d_ in enumerate(DT):
                t = kh.tile([P, NG * L], BF, tag="kh")
                nc.vector.tensor_mul(t[:d_], kts[ti][:d_], rk[:d_])
                khs.append(t)
            pc = psC.tile([L, NG * L], F32, tag="pc")
            for n in range(NG):
                sl = ts(n, L)
                for si, st in enumerate(DT):
                    nc.tensor.matmul(pc[:, sl], lhsT=khs[si][:st, sl],
                                     rhs=acps[si][:st, sl], start=(si == 0), stop=(si == 2))
            cab = cp.tile([L, NG * L], F32, tag="cab")
            nc.scalar.activation(cab[:], pc[:], AF.Abs)
            m1 = cp.tile([L, NG * L], F32, tag="m1")
            nc.vector.tensor_mul(m1[:], cab[:], ra[:L, :])
            nc.gpsimd.tensor_tensor(out=acc[:], in0=acc[:], in1=m1[:], op=OP.add)

        # fold 4 n-slices
        f1 = semp.tile([L, L], F32, tag="f1")
        nc.gpsimd.tensor_tensor(out=f1[:], in0=acc[:, ts(0, L)], in1=acc[:, ts(1, L)], op=OP.add)
        f2 = semp.tile([L, L], F32, tag="f2")
        nc.gpsimd.tensor_tensor(out=f2[:], in0=acc[:, ts(2, L)], in1=acc[:, ts(3, L)], op=OP.add)
        accb = semp.tile([L, L], F32, tag="accb")
        nc.gpsimd.tensor_tensor(out=accb[:], in0=f1[:], in1=f2[:], op=OP.add)

        # ------- semantic tail: score, windowed softmax, combine -------
        def st(tag, shape=(L, L), dt_=F32):
            return semp.tile(list(shape), dt_, tag=tag, name=tag)

        xc = st("xc")
        nc.vector.tensor_scalar(out=xc[:], in0=cos_sb[b][:], scalar1=CLIP,
                                scalar2=-CLIP, op0=OP.min, op1=OP.max)
        t_ = st("t")
        nc.scalar.activation(t_[:], xc[:], AF.Abs)
        t2 = st("t2")
        nc.vector.tensor_mul(t2[:], t_[:], t_[:])
        e_ = st("e")
        nc.vector.tensor_scalar(out=e_[:], in0=t2[:], scalar1=A2, scalar2=A0,
                                op0=OP.mult, op1=OP.add)
        o_ = st("o")
        nc.vector.tensor_scalar(out=o_[:], in0=t2[:], scalar1=A3, scalar2=A1,
                                op0=OP.mult, op1=OP.add)
        o2 = st("o2")
        nc.vector.tensor_mul(o2[:], o_[:], t_[:])
        pl = st("pl")
        nc.vector.tensor_add(pl[:], e_[:], o2[:])
        sm = st("sm")
        nc.scalar.activation(sm[:], t_[:], AF.Sqrt, bias=1.0, scale=-1.0)
        q_ = st("q")
        nc.vector.tensor_mul(q_[:], sm[:], pl[:])
        sg = st("sg")
        nc.scalar.sign(sg[:], xc[:])
        m_ = st("m")
        nc.vector.tensor_mul(m_[:], sg[:], q_[:])
        u_ = st("u")
        nc.vector.tensor_scalar(out=u_[:], in0=sg[:], scalar1=0.5, scalar2=0.5,
                                op0=OP.mult, op1=OP.add)
        v_ = st("v")
        nc.vector.tensor_scalar(out=v_[:], in0=m_[:], scalar1=-1.0 / math.pi,
                                scalar2=None, op0=OP.mult)
        sc_ = st("sc")
        nc.vector.tensor_add(sc_[:], u_[:], v_[:])
        s1 = st("s1")
        nc.vector.tensor_mul(s1[:], sc_[:], fm_sb[b][:])
        sM = st("sM")
        nc.vector.tensor_add(sM[:], s1[:], fneg_sb[b][:])
        mx = st("mx", (L, 1))
        nc.vector.tensor_reduce(out=mx[:], in_=sM[:], axis=AX.X, op=OP.max)
        nmx = st("nmx", (L, 1))
        nc.vector.tensor_scalar(out=nmx[:], in0=mx[:], scalar1=-1.0, scalar2=None,
                                op0=OP.mult)
        ex = st("ex")
        rsum = st("rsum", (L, 1))
        nc.scalar.activation(ex[:], sM[:], AF.Exp, bias=nmx[:], accum_out=rsum[:])
        rr = st("rr", (L, 1))
        nc.vector.reciprocal(rr[:], rsum[:])
        al = st("al")
        nc.vector.tensor_scalar(out=al[:], in0=ex[:], scalar1=rr[:], scalar2=None,
                                op0=OP.mult)
        c1 = st("c1")
        nc.vector.tensor_scalar(out=c1[:], in0=accb[:], scalar1=5.0, scalar2=None,
                                op0=OP.mult)
        c2 = st("c2")
        nc.vector.tensor_scalar(out=c2[:], in0=al[:], scalar1=0.5, scalar2=None,
                                op0=OP.mult)
        c3 = st("c3")
        nc.vector.tensor_add(c3[:], c1[:], c2[:])
        ob = st("ob")
        nc.vector.tensor_mul(ob[:], c3[:], fm_sb[b][:])
        nc.sync.dma_start(out=out[b], in_=ob[:])


_NC_CACHE = None


def _get_nc():
    global _NC_CACHE
    if _NC_CACHE is None:
        _NC_CACHE = _build_nc()
    return _NC_CACHE


def _make_in_maps(node_features, knowledge, weight_sem, weight_con, text_len):
    bf = ml_dtypes.bfloat16
    node_features = np.asarray(node_features, np.float32)
    knowledge = np.asarray(knowledge, np.float32)
    wsemT_ = np.ascontiguousarray(np.asarray(weight_sem, np.float32).T).astype(bf)
    wcon_ = np.ascontiguousarray(np.asarray(weight_con, np.float32)).astype(bf)
    tl = np.asarray(text_len).astype(np.int64)
    j = np.arange(L)[:, None]
    k = np.arange(L)[None, :]
    win = (k >= j - WP) & (k <= j + WF)
    in_maps = []
    for c in range(NCORES):
        sl = slice(c * BPC, (c + 1) * BPC)
        nf_nat = np.ascontiguousarray(node_features[sl])
        nfT = np.ascontiguousarray(node_features[sl].transpose(2, 0, 1)).astype(bf)
        kTp = np.ascontiguousarray(knowledge[sl].transpose(0, 3, 2, 1)).astype(bf)
        cur = tl[sl][:, None, None]
        fm = (win[None] & (k[None] <= cur - 1) & (j[None] < cur)).astype(np.float32)
        in_maps.append(dict(kT=kTp, nfT=nfT, nf=nf_nat, wsemT=wsemT_, wcon=wcon_,
                            fmask=np.ascontiguousarray(fm)))
    return in_maps


def run_on_hw(in_maps, trace=False, **kw):
    nc = _get_nc()
    return run_bass_kernel_spmd(nc, in_maps, list(range(NCORES)), trace=trace, **kw)


def kernel(node_features, knowledge, anew, weight_sem, weight_con, text_len):
    del anew  # strictly-positive affinity scale cancels in cosine similarity
    in_maps = _make_in_maps(node_features, knowledge, weight_sem, weight_con, text_len)
    res = run_on_hw(in_maps).results
    return np.concatenate([np.asarray(r["out"], np.float32) for r in res], axis=0)
